# revision 23
# baseline (speedup 1.0000x reference)
# Self-contained Trainium2 kernel for PVT-style sparse attention.
# reference: x[B=32,N=1280,C=256] -> q proj; conv-SR(2x2 s2)+LN on template(16x16)
# and search(32x32) tokens -> kv[320,C]; template->template-kv attention,
# search->all-kv attention; output proj. Batch-parallel over 8 NeuronCores.
import math
import sys
from contextlib import ExitStack

import numpy as np

sys.path.insert(0, "/opt/trn_rl_repo")

import ml_dtypes  # noqa: E402

import concourse.bass as bass  # noqa: E402
import concourse.mybir as mybir  # noqa: E402
import concourse.tile as tile  # noqa: E402
from concourse.bass_utils import run_bass_kernel_spmd  # noqa: E402

BF16 = mybir.dt.bfloat16
F32 = mybir.dt.float32

NCORES = 8
B = 32
BL = B // NCORES  # 4 batches per core
N = 1280
C = 256
H = 8
HD = 32
NT = 256   # template query tokens
MT = 64    # template kv tokens (8x8)
MS = 256   # search kv tokens (16x16)
M = MT + MS  # 320
SCALE = HD ** -0.5
LN_EPS = 1e-5

_CACHED = {}


def _build_nc():
    nc = bass.Bass()

    xT = nc.declare_dram_parameter("xT", [BL, C, N], BF16, isOutput=False)
    wpack = nc.declare_dram_parameter("wpack", [128, 16 * C], BF16, isOutput=False)
    bpack = nc.declare_dram_parameter("bpack", [1, 4 * C], BF16, isOutput=False)
    out = nc.declare_dram_parameter("out", [BL, N, C], F32, isOutput=True)

    EXP = mybir.ActivationFunctionType.Exp
    LN_ = mybir.ActivationFunctionType.Ln

    # ---- static SBUF ----
    def sb(name, shape, dt=BF16):
        return nc.alloc_sbuf_tensor(name, shape, dt).ap()

    wmega = sb("wmega", [128, 16 * C])
    bmega = sb("bmega", [1, 4 * C])
    ones_bf = sb("ones_bf", [128, 512])
    ones_f32 = sb("ones_f32", [1, 128], F32)
    eps_sb = sb("eps_sb", [1, 1], F32)

    def _blk(i):
        return wmega[:, i * C:(i + 1) * C]
    wq_sb = [_blk(0), _blk(1)]
    wk_sb = [_blk(2), _blk(3)]
    wv_sb = [_blk(4), _blk(5)]
    wp_sb = [_blk(6), _blk(7)]
    wsr_sb = {(kh, kw, ct): _blk(8 + kh * 4 + kw * 2 + ct)
              for kh in range(2) for kw in range(2) for ct in range(2)}
    uk_sb = bmega[:, 0:C]
    uv_sb = bmega[:, C:2 * C]
    srb_sb = bmega[:, 2 * C:3 * C]
    pb_sb = bmega[:, 3 * C:4 * C]

    xt_ = [[sb(f"xt{b}_{ct}", [128, N]) for ct in range(2)] for b in range(2)]
    qt_ = [[sb(f"qt{b}_{dt}", [128, N]) for dt in range(2)] for b in range(2)]
    yt_ = [[sb(f"yt{b}_{o}", [128, M]) for o in range(2)] for b in range(2)]
    ysq_ = [[sb(f"ysq{b}_{o}", [128, M]) for o in range(2)] for b in range(2)]
    yn_ = [[sb(f"yn{b}_{o}", [128, M]) for o in range(2)] for b in range(2)]
    kt_ = [[sb(f"kt{b}_{d}", [128, M]) for d in range(2)] for b in range(2)]
    v_ = [[sb(f"v{b}_{m}", [128, C]) for m in range(3)] for b in range(2)]
    et_ = [[sb(f"et{b}_{i}", [128, 512]) for i in range(10)] for b in range(2)]
    ot_ = [[sb(f"ot{b}_{d}", [128, N]) for d in range(2)] for b in range(2)]
    osb_ = [sb(f"osb{j}", [128, C], F32) for j in range(4)]
    mu_ = [sb(f"mu{b}", [1, M], F32) for b in range(2)]
    e2_ = [sb(f"e2{b}", [1, M], F32) for b in range(2)]
    var_ = [sb(f"var{b}", [1, M], F32) for b in range(2)]
    lnv_ = [sb(f"lnv{b}", [1, M], F32) for b in range(2)]
    rstd_ = [sb(f"rstd{b}", [1, M], F32) for b in range(2)]
    tmp_f = sb("tmp_f", [128, M], F32)
    rinv_sb = sb("rinv_sb", [128, 512], F32)
    rinv_tmp = sb("rinv_tmp", [128, 512], F32)

    banks = [nc.alloc_psum_tensor(f"bank{i}", [128, 512], F32).ap()
             for i in range(8)]

    QCH = [(0, 512), (512, 512), (1024, 256)]
    ACH = [(0, 256, True), (256, 512, False), (768, 512, False)]
    MTILES = [(0, 64), (64, 128), (192, 128)]

    # ---- plan: four engine streams, monotonic counters ----
    # DMA completions are NOT ordered across dma_starts (multi-queue), so a
    # single shared dma counter is racy: a later DMA's +16 can satisfy a
    # wait meant for an earlier one. Each independently-consumed resource
    # gets its own semaphore; with at most K uses issued when a consumer
    # waits >= 16*K, any increments summing to the threshold imply the
    # needed DMA completed.
    DMA_SEMS = ["dma_c", "dma_x0", "dma_x1", "dma_o0", "dma_o1", "dma_o2",
                "dma_o3"]
    plan = {"sync": [], "pe": [], "dve": [], "act": []}
    cnt = {"pe": 0, "dve": 0, "act": 0}
    cnt.update({s: 0 for s in DMA_SEMS})
    last_bank = {i: None for i in range(8)}  # bank -> (sem, val) last consumer
    osb_last = {j: None for j in range(4)}

    def op(engine, fn, inc=None):
        # fn(eng) -> instruction (or None); inc sem applied via then_inc
        plan[engine].append(("op", fn, inc))
        if inc is not None:
            cnt[inc] += 16 if inc in DMA_SEMS else 1
        return cnt[inc] if inc else None

    def wait(engine, sem_name, val):
        if val is None or val <= 0:
            return
        plan[engine].append(("wait", sem_name, val))

    def wait_bank(engine, i):
        if last_bank[i] is not None:
            s, v = last_bank[i]
            if s != engine:
                wait(engine, s, v)

    # const DMAs + memsets
    op("sync", lambda e: e.dma_start(out=wmega, in_=wpack[:, :]), "dma_c")
    CONST_DMA = op("sync", lambda e: e.dma_start(out=bmega, in_=bpack[:, :]),
                   "dma_c")
    op("dve", lambda e: e.memset(ones_bf, 1.0), "dve")
    op("dve", lambda e: e.memset(ones_f32, 1.0), "dve")
    MEMSET_DONE = op("dve", lambda e: e.memset(eps_sb, LN_EPS), "dve")

    pe_conv_mark = {}
    rinv_last = [None]
    dma_xt_mark = {}
    ysq_done_mark = {}
    kv_done_mark = {}
    att_done_mark = {}

    def emit_proj(bl):
        b = bl % 2
        wait("pe", "dve", att_done_mark[bl])
        for nt in range(10):
            bank = nt % 2
            wait_bank("pe", bank)
            op("pe", lambda e, bank=bank: nc.tensor.matmul(
                banks[bank][:, 0:C], ones_bf[0:1, 0:128], pb_sb[0:1, :],
                start=True, stop=False, skip_group_check=True), None)
            for dt in range(2):
                op("pe", lambda e, bank=bank, nt=nt, dt=dt, b=b:
                   nc.tensor.matmul(
                       banks[bank][:, 0:C],
                       ot_[b][dt][:, nt * 128:(nt + 1) * 128], wp_sb[dt],
                       start=False, stop=(dt == 1), skip_group_check=True),
                   "pe" if dt == 1 else None)
            m = cnt["pe"]
            j = nt % 4
            wait("dve", "pe", m)
            if osb_last[j] is not None:
                wait("dve", f"dma_o{j}", osb_last[j])
            cv = op("dve", lambda e, bank=bank, j=j:
                    e.tensor_copy(osb_[j], banks[bank][:, 0:C]), "dve")
            last_bank[bank] = ("dve", cv)
            wait("sync", "dve", cv)
            dv = op("sync", lambda e, j=j, nt=nt, bl=bl: e.dma_start(
                out=out[bl, nt * 128:(nt + 1) * 128, :], in_=osb_[j]),
                f"dma_o{j}")
            osb_last[j] = dv


    def emit_load_qconv(bl):
        b = bl % 2
        # ---- xT load ----
        xsem = f"dma_x{b}"
        if bl >= 2:
            wait("sync", "pe", pe_conv_mark[bl - 2])
        for ct in range(2):
            op("sync", lambda e, ct=ct, b=b, bl=bl: e.dma_start(
                out=xt_[b][ct], in_=xT[bl, ct * 128:(ct + 1) * 128, :]), xsem)
        dma_xt_mark[bl] = cnt[xsem]

        # ---- qT ----
        wait("pe", xsem, dma_xt_mark[bl])
        if bl == 0:
            wait("pe", "dma_c", CONST_DMA)
            wait("pe", "dve", MEMSET_DONE)
        for g, (dt, (n0, w)) in enumerate(
                [(dt, ch) for dt in range(2) for ch in QCH]):
            bank = g % 2
            wait_bank("pe", bank)
            for ct in range(2):
                op("pe", lambda e, bank=bank, ct=ct, dt=dt, n0=n0, w=w, b=b:
                   nc.tensor.matmul(
                       banks[bank][:, :w],
                       wq_sb[ct][:, dt * 128:(dt + 1) * 128],
                       xt_[b][ct][:, n0:n0 + w],
                       start=(ct == 0), stop=(ct == 1),
                       skip_group_check=True),
                   "pe" if ct == 1 else None)
            m = cnt["pe"]
            wait("dve", "pe", m)
            cv = op("dve", lambda e, bank=bank, dt=dt, n0=n0, w=w, b=b:
                    e.tensor_copy(qt_[b][dt][:, n0:n0 + w], banks[bank][:, :w]),
                    "dve")
            last_bank[bank] = ("dve", cv)

        # ---- conv ----
        for ot in range(2):
            bank = ot % 2
            wait_bank("pe", bank)
            op("pe", lambda e, bank=bank, ot=ot: nc.tensor.matmul(
                banks[bank][:, 0:M],
                srb_sb[0:1, ot * 128:(ot + 1) * 128],
                ones_bf[0:1, 0:M],
                start=True, stop=False, skip_group_check=True), None)
            idx = 0
            for kh in range(2):
                for kw in range(2):
                    for ct in range(2):
                        idx += 1
                        op("pe", lambda e, bank=bank, ot=ot, kh=kh, kw=kw,
                           ct=ct, b=b: nc.tensor.matmul(
                               banks[bank][:, 0:MT],
                               wsr_sb[(kh, kw, ct)][:, ot * 128:(ot + 1) * 128],
                               xt_[b][ct][:, 0:256].rearrange(
                                   "p (i a j b2) -> p a b2 i j",
                                   i=8, a=2, j=8, b2=2)[:, kh, kw, :, :],
                               start=False, stop=False,
                               skip_group_check=True), None)
                        op("pe", lambda e, bank=bank, ot=ot, kh=kh, kw=kw,
                           ct=ct, b=b, idx=idx: nc.tensor.matmul(
                               banks[bank][:, MT:M],
                               wsr_sb[(kh, kw, ct)][:, ot * 128:(ot + 1) * 128],
                               xt_[b][ct][:, 256:1280].rearrange(
                                   "p (i a j b2) -> p a b2 i j",
                                   i=16, a=2, j=16, b2=2)[:, kh, kw, :, :],
                               start=False, stop=(idx == 8),
                               skip_group_check=True),
                           "pe" if idx == 8 else None)
            m = cnt["pe"]
            if ot == 1:
                pe_conv_mark[bl] = m
            wait("dve", "pe", m)
            cv = op("dve", lambda e, bank=bank, ot=ot, b=b:
                    e.tensor_copy(yt_[b][ot], banks[bank][:, 0:M]), "dve")
            last_bank[bank] = ("dve", cv)
            wait("dve", "dve", cv)  # self-sync: drain yt write before read
            op("dve", lambda e, ot=ot, b=b:
               e.tensor_mul(ysq_[b][ot], yt_[b][ot], yt_[b][ot]), "dve")
        ysq_done_mark[bl] = cnt["dve"]
        YSQ_DONE = cnt["dve"]


    emit_load_qconv(0)
    for bl in range(BL):
        b = bl % 2
        # ---- LN stats ----
        wait("pe", "dve", ysq_done_mark[bl])
        wait_bank("pe", 6)
        wait_bank("pe", 7)
        for ot in range(2):
            op("pe", lambda e, ot=ot, b=b: nc.tensor.matmul(
                banks[6][0:1, 0:M], ones_bf[:, 0:1], yt_[b][ot],
                start=(ot == 0), stop=(ot == 1), skip_group_check=True),
               "pe" if ot == 1 else None)
        for ot in range(2):
            op("pe", lambda e, ot=ot, b=b: nc.tensor.matmul(
                banks[7][0:1, 0:M], ones_bf[:, 0:1], ysq_[b][ot],
                start=(ot == 0), stop=(ot == 1), skip_group_check=True),
               "pe" if ot == 1 else None)
        PE_STATS = cnt["pe"]
        wait("dve", "pe", PE_STATS)
        op("dve", lambda e, b=b: e.tensor_single_scalar(
            out=mu_[b], in_=banks[6][0:1, 0:M], scalar=1.0 / C,
            op=mybir.AluOpType.mult), "dve")
        sv = op("dve", lambda e, b=b: e.tensor_single_scalar(
            out=e2_[b], in_=banks[7][0:1, 0:M], scalar=1.0 / C,
            op=mybir.AluOpType.mult), "dve")
        wait("dve", "dve", sv)
        sv = op("dve", lambda e, b=b: e.tensor_mul(var_[b], mu_[b], mu_[b]),
                "dve")
        wait("dve", "dve", sv)
        op("dve", lambda e, b=b: e.tensor_sub(var_[b], e2_[b], var_[b]), "dve")
        DVE_VAR = cnt["dve"]
        wait("act", "dve", DVE_VAR)
        av_ = op("act", lambda e, b=b: e.activation(lnv_[b], var_[b], LN_,
                                                    bias=eps_sb[0:1, 0:1]),
                 "act")
        wait("act", "act", av_)
        op("act", lambda e, b=b: e.activation(rstd_[b], lnv_[b], EXP,
                                              scale=-0.5), "act")
        ACT_RSTD = cnt["act"]

        if bl > 0:
            emit_proj(bl - 1)

        # broadcast mu/rstd (banks 6/7 freed by ACT_MU/E2 reads)
        wait("pe", "act", ACT_RSTD)
        op("pe", lambda e, b=b: nc.tensor.matmul(
            banks[6][:, 0:M], ones_f32[0:1, :], mu_[b],
            start=True, stop=True, skip_group_check=True), "pe")
        op("pe", lambda e, b=b: nc.tensor.matmul(
            banks[7][:, 0:M], ones_f32[0:1, :], rstd_[b],
            start=True, stop=True, skip_group_check=True), "pe")
        PE_BCAST = cnt["pe"]
        wait("dve", "pe", PE_BCAST)
        for ot in range(2):
            if ot == 1:
                wait("dve", "dve", cnt["dve"])  # WAR: tmp_f reread pending
            sv = op("dve", lambda e, ot=ot, b=b:
                    e.tensor_sub(tmp_f, yt_[b][ot], banks[6][:, 0:M]), "dve")
            wait("dve", "dve", sv)
            op("dve", lambda e, ot=ot, b=b:
               e.tensor_mul(yn_[b][ot], tmp_f, banks[7][:, 0:M]), "dve")
        DVE_NORM = cnt["dve"]
        last_bank[6] = ("dve", DVE_NORM)
        last_bank[7] = ("dve", DVE_NORM)

        # ---- kT ----
        wait("pe", "dve", DVE_NORM)
        for dt in range(2):
            bank = dt % 2
            wait_bank("pe", bank)
            op("pe", lambda e, bank=bank, dt=dt: nc.tensor.matmul(
                banks[bank][:, 0:M],
                uk_sb[0:1, dt * 128:(dt + 1) * 128], ones_bf[0:1, 0:M],
                start=True, stop=False, skip_group_check=True), None)
            for ct in range(2):
                op("pe", lambda e, bank=bank, dt=dt, ct=ct, b=b:
                   nc.tensor.matmul(
                       banks[bank][:, 0:M],
                       wk_sb[ct][:, dt * 128:(dt + 1) * 128], yn_[b][ct],
                       start=False, stop=(ct == 1), skip_group_check=True),
                   "pe" if ct == 1 else None)
            m = cnt["pe"]
            wait("dve", "pe", m)
            cv = op("dve", lambda e, bank=bank, dt=dt, b=b:
                    e.tensor_copy(kt_[b][dt], banks[bank][:, 0:M]), "dve")
            last_bank[bank] = ("dve", cv)

        # ---- V ----
        for mi, (m0, mr) in enumerate(MTILES):
            bank = mi % 2
            wait_bank("pe", bank)
            op("pe", lambda e, bank=bank, mr=mr: nc.tensor.matmul(
                banks[bank][:mr, 0:C], ones_bf[0:1, 0:mr], uv_sb[0:1, :],
                start=True, stop=False, skip_group_check=True), None)
            for ct in range(2):
                op("pe", lambda e, bank=bank, m0=m0, mr=mr, ct=ct, b=b:
                   nc.tensor.matmul(
                       banks[bank][:mr, 0:C],
                       yn_[b][ct][:, m0:m0 + mr], wv_sb[ct],
                       start=False, stop=(ct == 1), skip_group_check=True),
                   "pe" if ct == 1 else None)
            m = cnt["pe"]
            wait("dve", "pe", m)
            cv = op("dve", lambda e, bank=bank, mi=mi, mr=mr, b=b:
                    e.tensor_copy(v_[b][mi][:mr, :], banks[bank][:mr, 0:C]),
                    "dve")
            if mi == 0:
                cv = op("dve", lambda e, bank=bank, b=b:
                        e.tensor_copy(v_[b][0][64:128, :],
                                      banks[bank][0:64, 0:C]), "dve")
            last_bank[bank] = ("dve", cv)

        kv_done_mark[bl] = cnt["dve"]
        if bl + 1 < BL:
            emit_load_qconv(bl + 1)

        # ---- attention ----
        for (n0, w, is_t) in ACH:
            for dt in range(2):
                tiles = [("T", 0), ("T", 1)]
                if not is_t:
                    tiles += [(mi, h4) for mi in (1, 2) for h4 in range(4)]
                exp_marks = []
                for ti, (kind, sub) in enumerate(tiles):
                    bank = 2 + (ti % 2)
                    wait_bank("pe", bank)
                    if ti == 0:
                        wait("pe", "dve", kv_done_mark[bl])
                    if kind == "T":
                        pair = sub
                        for z, h4 in enumerate((2 * pair, 2 * pair + 1)):
                            op("pe", lambda e, bank=bank, h4=h4, z=z, n0=n0,
                               w=w, dt=dt, b=b: nc.tensor.matmul(
                                   banks[bank][64 * z:64 * z + 64, :w],
                                   kt_[b][dt][32 * h4:32 * h4 + 32, 0:64],
                                   qt_[b][dt][32 * h4:32 * h4 + 32, n0:n0 + w],
                                   start=True, stop=True,
                                   skip_group_check=True,
                                   tile_position=(32 * h4, 64 * z)),
                               "pe" if z == 1 else None)
                    else:
                        mi, h4 = kind, sub
                        m0m, mrm = MTILES[mi]
                        op("pe", lambda e, bank=bank, h4=h4, m0m=m0m, mrm=mrm,
                           n0=n0, w=w, dt=dt, b=b: nc.tensor.matmul(
                               banks[bank][:mrm, :w],
                               kt_[b][dt][32 * h4:32 * h4 + 32, m0m:m0m + mrm],
                               qt_[b][dt][32 * h4:32 * h4 + 32, n0:n0 + w],
                               start=True, stop=True, skip_group_check=True,
                               tile_position=(32 * h4, 0)), "pe")
                    m = cnt["pe"]
                    wait("act", "pe", m)
                    ev = op("act", lambda e, bank=bank, ti=ti, w=w, b=b:
                            e.activation(et_[b][ti][:, :w], banks[bank][:, :w],
                                         EXP, scale=SCALE), "act")
                    exp_marks.append(ev)
                    last_bank[bank] = ("act", ev)

                # AV + sums
                wait("pe", "act", exp_marks[-1])
                wait_bank("pe", 4)
                wait_bank("pe", 5)
                for h4 in range(4):
                    h = 4 * dt + h4
                    pair, lohi = divmod(h4, 2)
                    pb_ = 64 * lohi
                    ti_t = pair
                    op("pe", lambda e, h4=h4, h=h, pb_=pb_, ti_t=ti_t, w=w,
                       is_t=is_t, b=b: nc.tensor.matmul(
                           banks[4][32 * h4:32 * h4 + 32, :w],
                           v_[b][0][pb_:pb_ + 64, h * 32:h * 32 + 32],
                           et_[b][ti_t][pb_:pb_ + 64, :w],
                           start=True, stop=is_t, skip_group_check=True,
                           tile_position=(pb_, 32 * h4)), None)
                    op("pe", lambda e, h4=h4, pb_=pb_, ti_t=ti_t, w=w,
                       is_t=is_t, b=b: nc.tensor.matmul(
                           banks[5][32 * h4:32 * h4 + 32, :w],
                           ones_bf[pb_:pb_ + 64, 0:32],
                           et_[b][ti_t][pb_:pb_ + 64, :w],
                           start=True, stop=is_t, skip_group_check=True,
                           tile_position=(pb_, 32 * h4)),
                       "pe" if (is_t and h4 == 3) else None)
                    if not is_t:
                        for mi in (1, 2):
                            m0m, mrm = MTILES[mi]
                            ti_s = 2 + (mi - 1) * 4 + h4
                            op("pe", lambda e, h4=h4, h=h, mi=mi, mrm=mrm,
                               ti_s=ti_s, w=w, b=b: nc.tensor.matmul(
                                   banks[4][32 * h4:32 * h4 + 32, :w],
                                   v_[b][mi][:mrm, h * 32:h * 32 + 32],
                                   et_[b][ti_s][:mrm, :w],
                                   start=False, stop=(mi == 2),
                                   skip_group_check=True,
                                   tile_position=(0, 32 * h4)), None)
                            op("pe", lambda e, h4=h4, mi=mi, mrm=mrm,
                               ti_s=ti_s, w=w, b=b: nc.tensor.matmul(
                                   banks[5][32 * h4:32 * h4 + 32, :w],
                                   ones_bf[:mrm, 0:32],
                                   et_[b][ti_s][:mrm, :w],
                                   start=False, stop=(mi == 2),
                                   skip_group_check=True,
                                   tile_position=(0, 32 * h4)),
                               "pe" if (h4 == 3 and mi == 2) else None)
                PE_AVS = cnt["pe"]
                wait("dve", "pe", PE_AVS)
                rv = op("dve", lambda e, w=w: e.reciprocal(
                    rinv_sb[:, :w], banks[5][:, :w]), "dve")
                last_bank[5] = ("dve", rv)
                wait("dve", "dve", rv)
                mvv = op("dve", lambda e, dt=dt, n0=n0, w=w, b=b:
                         e.tensor_mul(ot_[b][dt][:, n0:n0 + w],
                                      banks[4][:, :w], rinv_sb[:, :w]), "dve")
                last_bank[4] = ("dve", mvv)

        att_done_mark[bl] = cnt["dve"]
        if bl == BL - 1:
            emit_proj(bl)

    # ---- emit ----
    with ExitStack() as stack:
        block = stack.enter_context(nc.Block())
        sems = {}
        for name in ["pe", "dve", "act"] + DMA_SEMS:
            sems[name] = stack.enter_context(nc.semaphore(f"{name}_sem"))

        def emit(eng_obj, items):
            for kind, a, b2 in items:
                if kind == "wait":
                    eng_obj.wait_ge(sems[a], b2)
                else:
                    ins = a(eng_obj)
                    if b2 is not None:
                        ins.then_inc(sems[b2], 16 if b2 in DMA_SEMS else 1)

        @block.sync
        def _(sync):
            emit(sync, plan["sync"])

        @block.tensor
        def _(tensor):
            emit(tensor, plan["pe"])

        @block.vector
        def _(vector):
            emit(vector, plan["dve"])

        @block.scalar
        def _(scalar):
            emit(scalar, plan["act"])

    return nc


def _get_nc():
    if "nc" not in _CACHED:
        _CACHED["nc"] = _build_nc()
    return _CACHED["nc"]


def _bf16(a):
    return np.ascontiguousarray(a.astype(ml_dtypes.bfloat16))


def kernel(x, Wq, Wkv, sr_w, sr_b, ln_g, ln_b, proj_w, proj_b,
           t_h=16, t_w=16, s_h=32, s_w=32, **_ignored):
    x = np.asarray(x, np.float32)
    Wq = np.asarray(Wq, np.float32)
    Wkv = np.asarray(Wkv, np.float32)
    sr_w = np.asarray(sr_w, np.float32)
    sr_b = np.asarray(sr_b, np.float32)
    ln_g = np.asarray(ln_g, np.float32)
    ln_b = np.asarray(ln_b, np.float32)
    proj_w = np.asarray(proj_w, np.float32)
    proj_b = np.asarray(proj_b, np.float32)

    # host-side prep: transposes + LN-gamma/beta folding (cheap, numpy)
    xT = _bf16(x.transpose(0, 2, 1))                      # [B, C, N]
    wqT = Wq.T                                            # [c, d]
    wsr = sr_w.transpose(2, 3, 1, 0)                      # [kh, kw, c, o]
    wkT = (Wkv[:C] * ln_g[None, :]).T
    wvT = (Wkv[C:] * ln_g[None, :]).T
    uk = (Wkv[:C] @ ln_b)[None, :]
    uv = (Wkv[C:] @ ln_b)[None, :]
    srb = sr_b[None, :]
    wpT = proj_w.T
    pb = proj_b[None, :]
    blocks = [wqT[:128], wqT[128:], wkT[:128], wkT[128:],
              wvT[:128], wvT[128:], wpT[:128], wpT[128:]]
    for kh in range(2):
        for kw in range(2):
            for ct in range(2):
                blocks.append(wsr[kh, kw, ct * 128:(ct + 1) * 128, :])
    wpack = _bf16(np.concatenate(blocks, axis=1))         # [128, 16*C]
    bpack = _bf16(np.concatenate([uk, uv, srb, pb], axis=1))  # [1, 4*C]

    nc = _get_nc()
    in_maps = []
    for c in range(NCORES):
        in_maps.append({
            "xT": xT[c * BL:(c + 1) * BL],
            "wpack": wpack, "bpack": bpack,
        })
    res = run_bass_kernel_spmd(nc, in_maps, core_ids=list(range(NCORES)))
    _CACHED["last_results"] = res
    outs = [np.asarray(r["out"]).astype(np.float32) for r in res.results]
    return np.concatenate(outs, axis=0)



# revision 39
# speedup vs baseline: 1.5798x; 1.5798x over previous
# Self-contained Trainium2 kernel for PVT-style sparse attention.
# reference: x[B=32,N=1280,C=256] -> q proj; conv-SR(2x2 s2)+LN on template(16x16)
# and search(32x32) tokens -> kv[320,C]; template->template-kv attention,
# search->all-kv attention; output proj. Batch-parallel over 8 NeuronCores.
import math
import sys
from contextlib import ExitStack

import numpy as np

sys.path.insert(0, "/opt/trn_rl_repo")

import ml_dtypes  # noqa: E402

import concourse.bass as bass  # noqa: E402
import concourse.mybir as mybir  # noqa: E402
import concourse.tile as tile  # noqa: E402
from concourse.bass_utils import run_bass_kernel_spmd  # noqa: E402

BF16 = mybir.dt.bfloat16
F32 = mybir.dt.float32

NCORES = 8
B = 32
BL = B // NCORES  # 4 batches per core
N = 1280
C = 256
H = 8
HD = 32
NT = 256   # template query tokens
MT = 64    # template kv tokens (8x8)
MS = 256   # search kv tokens (16x16)
M = MT + MS  # 320
SCALE = HD ** -0.5
LN_EPS = 1e-5

_CACHED = {}


def _build_nc():
    nc = bass.Bass()

    xT = nc.declare_dram_parameter("xT", [BL, C, N], BF16, isOutput=False)
    wpack = nc.declare_dram_parameter("wpack", [128, 17 * C], BF16, isOutput=False)
    bpack = nc.declare_dram_parameter("bpack", [1, 4 * C], BF16, isOutput=False)
    out = nc.declare_dram_parameter("out", [BL, N, C], F32, isOutput=True)

    EXP = mybir.ActivationFunctionType.Exp
    LN_ = mybir.ActivationFunctionType.Ln

    # ---- static SBUF ----
    def sb(name, shape, dt=BF16):
        return nc.alloc_sbuf_tensor(name, shape, dt).ap()

    wmega = sb("wmega", [128, 17 * C])
    pb_tile = wmega[:, 16 * C:17 * C]
    bmega = sb("bmega", [1, 4 * C])
    ones_bf = sb("ones_bf", [128, 512])
    ones_f32 = sb("ones_f32", [1, 128], F32)
    eps_sb = sb("eps_sb", [1, 1], F32)

    def _blk(i):
        return wmega[:, i * C:(i + 1) * C]
    wq_sb = [_blk(0), _blk(1)]
    wk_sb = [_blk(2), _blk(3)]
    wv_sb = [_blk(4), _blk(5)]
    wp_sb = [_blk(6), _blk(7)]
    wsr_sb = {(kh, kw, ct): _blk(8 + kh * 4 + kw * 2 + ct)
              for kh in range(2) for kw in range(2) for ct in range(2)}
    uk_sb = bmega[:, 0:C]
    uv_sb = bmega[:, C:2 * C]
    srb_sb = bmega[:, 2 * C:3 * C]
    pb_sb = bmega[:, 3 * C:4 * C]

    xt_ = [[sb(f"xt{b}_{ct}", [128, N]) for ct in range(2)] for b in range(2)]
    qt_ = [[sb(f"qt{b}_{dt}", [128, N]) for dt in range(2)] for b in range(2)]
    yt_ = [[sb(f"yt{b}_{o}", [128, M]) for o in range(2)] for b in range(2)]
    ysq_ = [[sb(f"ysq{b}_{o}", [128, M]) for o in range(2)] for b in range(2)]
    yn_ = [[sb(f"yn{b}_{o}", [128, M]) for o in range(2)] for b in range(2)]
    kt_ = [[sb(f"kt{b}_{d}", [128, M]) for d in range(2)] for b in range(2)]
    v_ = [[sb(f"v{b}_{m}", [128, C]) for m in range(3)] for b in range(2)]
    etp_ = [[sb(f"etp{b}_{p}", [128, 1024]) for p in range(5)]
            for b in range(2)]
    et_ = [[etp_[b][i // 2][:, (i % 2) * 512:(i % 2) * 512 + 512]
            for i in range(10)] for b in range(2)]
    ot_ = [[sb(f"ot{b}_{d}", [128, N]) for d in range(2)] for b in range(2)]
    osb_ = [sb(f"osb{j}", [128, C], F32) for j in range(4)]
    mu_ = [sb(f"mu{b}", [1, M], F32) for b in range(2)]
    e2_ = [sb(f"e2{b}", [1, M], F32) for b in range(2)]
    var_ = [sb(f"var{b}", [1, M], F32) for b in range(2)]
    lnv_ = [sb(f"lnv{b}", [1, M], F32) for b in range(2)]
    rstd_ = [sb(f"rstd{b}", [1, M], F32) for b in range(2)]
    tmp_f = sb("tmp_f", [128, M], F32)
    av_stage = [sb(f"avst{u}", [128, 1024], F32) for u in range(2)]
    rinv_sb = sb("rinv_sb", [128, 512], F32)
    rinv_tmp = sb("rinv_tmp", [128, 512], F32)

    # banks 2,3 and 6,7 are views of two-bank pair tensors so one ACT
    # instruction can exp two S^T tiles at once ([128, 2, w] AP).
    bank0 = nc.alloc_psum_tensor("bank0", [128, 512], F32).ap()
    bank1 = nc.alloc_psum_tensor("bank1", [128, 512], F32).ap()
    pairA = nc.alloc_psum_tensor("pairA", [128, 1024], F32).ap()
    pairC = nc.alloc_psum_tensor("pairC", [128, 1024], F32).ap()
    pairB = nc.alloc_psum_tensor("pairB", [128, 1024], F32).ap()
    banks = [bank0, bank1, pairA[:, 0:512], pairA[:, 512:1024],
             pairC[:, 0:512], pairC[:, 512:1024],
             pairB[:, 0:512], pairB[:, 512:1024]]
    pair_of = {0: pairA, 1: pairB}

    QCH = [(0, 512), (512, 512), (1024, 256)]
    ACH = [(0, 256, True), (256, 512, False), (768, 512, False)]
    MTILES = [(0, 64), (64, 128), (192, 128)]

    # ---- plan: four engine streams, monotonic counters ----
    # DMA completions are NOT ordered across dma_starts (multi-queue), so a
    # single shared dma counter is racy: a later DMA's +16 can satisfy a
    # wait meant for an earlier one. Each independently-consumed resource
    # gets its own semaphore; with at most K uses issued when a consumer
    # waits >= 16*K, any increments summing to the threshold imply the
    # needed DMA completed.
    DMA_SEMS = ["dma_c", "dma_x0", "dma_x1", "dma_o0", "dma_o1", "dma_o2",
                "dma_o3"]
    plan = {"sync": [], "pe": [], "dve": [], "act": []}
    cnt = {"pe": 0, "dve": 0, "act": 0}
    cnt.update({s: 0 for s in DMA_SEMS})
    last_bank = {i: None for i in range(8)}  # bank -> (sem, val) last consumer
    osb_last = {j: None for j in range(4)}

    def op(engine, fn, inc=None):
        # fn(eng) -> instruction (or None); inc sem applied via then_inc
        plan[engine].append(("op", fn, inc))
        if inc is not None:
            cnt[inc] += 16 if inc in DMA_SEMS else 1
        return cnt[inc] if inc else None

    def wait(engine, sem_name, val):
        if val is None or val <= 0:
            return
        plan[engine].append(("wait", sem_name, val))

    def wait_bank(engine, i):
        if last_bank[i] is not None:
            s, v = last_bank[i]
            if s != engine:
                wait(engine, s, v)

    # const DMAs + memsets
    op("sync", lambda e: e.dma_start(out=wmega, in_=wpack[:, :]), "dma_c")
    CONST_DMA = op("sync", lambda e: e.dma_start(out=bmega, in_=bpack[:, :]),
                   "dma_c")
    op("dve", lambda e: e.memset(ones_bf, 1.0), "dve")
    op("dve", lambda e: e.memset(ones_f32, 1.0), "dve")
    MEMSET_DONE = op("dve", lambda e: e.memset(eps_sb, LN_EPS), "dve")

    pe_conv_mark = {}
    unit_ctr = [0]
    av_last = {0: None, 1: None}
    rinv_last = [None]
    dma_xt_mark = {}
    ysq_done_mark = {}
    kv_done_mark = {}
    kt_done_mark = {}
    act_rstd_mark = {}
    dve_norm_mark = {}
    att_chunk_mark = {}
    att_done_mark = {}

    def proj_parcels(bl):
        b = bl % 2
        parcels = []

        def tile(nt, first):
            bank = nt % 2
            ci = 0 if nt < 2 else (1 if nt < 6 else 2)
            wait("pe", "dve", att_chunk_mark[(bl, ci)])
            wait_bank("pe", bank)
            for dt in range(2):
                op("pe", lambda e, bank=bank, nt=nt, dt=dt, b=b:
                   nc.tensor.matmul(
                       banks[bank][:, 0:C],
                       ot_[b][dt][:, nt * 128:(nt + 1) * 128], wp_sb[dt],
                       start=(dt == 0), stop=(dt == 1), skip_group_check=True),
                   "pe" if dt == 1 else None)
            m = cnt["pe"]
            j = nt % 4
            wait("dve", "pe", m)
            if osb_last[j] is not None:
                wait("dve", f"dma_o{j}", osb_last[j])
            # bias add fused into the PSUM->SBUF copy
            cv = op("dve", lambda e, bank=bank, j=j: e.tensor_tensor(
                out=osb_[j], in0=banks[bank][:, 0:C], in1=pb_tile,
                op=mybir.AluOpType.add), "dve")
            last_bank[bank] = ("dve", cv)
            wait("sync", "dve", cv)
            dv = op("sync", lambda e, j=j, nt=nt, bl=bl: e.dma_start(
                out=out[bl, nt * 128:(nt + 1) * 128, :], in_=osb_[j]),
                f"dma_o{j}")
            osb_last[j] = dv

        for nt in range(10):
            parcels.append(lambda nt=nt: tile(nt, nt == 0))
        return parcels

    def emit_xt_dma(bl):
        b = bl % 2
        xsem = f"dma_x{b}"
        if bl >= 2:
            wait("sync", "pe", pe_conv_mark[bl - 2])
        for ct in range(2):
            op("sync", lambda e, ct=ct, b=b, bl=bl: e.dma_start(
                out=xt_[b][ct], in_=xT[bl, ct * 128:(ct + 1) * 128, :]), xsem)
        dma_xt_mark[bl] = cnt[xsem]

    def qt_parcel(bl, g):
        b = bl % 2
        dt, (n0, w) = [(dt, ch) for dt in range(2) for ch in QCH][g]
        wait("pe", f"dma_x{b}", dma_xt_mark[bl])
        bank = g % 2
        wait_bank("pe", bank)
        for ct in range(2):
            op("pe", lambda e, bank=bank, ct=ct, dt=dt, n0=n0, w=w, b=b:
               nc.tensor.matmul(
                   banks[bank][:, :w],
                   wq_sb[ct][:, dt * 128:(dt + 1) * 128],
                   xt_[b][ct][:, n0:n0 + w],
                   start=(ct == 0), stop=(ct == 1),
                   skip_group_check=True),
               "pe" if ct == 1 else None)
        m = cnt["pe"]
        wait("dve", "pe", m)
        cv = op("dve", lambda e, bank=bank, dt=dt, n0=n0, w=w, b=b:
                e.tensor_copy(qt_[b][dt][:, n0:n0 + w], banks[bank][:, :w]),
                "dve")
        last_bank[bank] = ("dve", cv)

    def conv_parcel(bl, ot):
        b = bl % 2
        wait("pe", f"dma_x{b}", dma_xt_mark[bl])
        bank = ot % 2
        wait_bank("pe", bank)
        op("pe", lambda e, bank=bank, ot=ot: nc.tensor.matmul(
            banks[bank][:, 0:M],
            srb_sb[0:1, ot * 128:(ot + 1) * 128],
            ones_bf[0:1, 0:M],
            start=True, stop=False, skip_group_check=True), None)
        idx = 0
        for kh in range(2):
            for kw in range(2):
                for ct in range(2):
                    idx += 1
                    op("pe", lambda e, bank=bank, ot=ot, kh=kh, kw=kw,
                       ct=ct, b=b: nc.tensor.matmul(
                           banks[bank][:, 0:MT],
                           wsr_sb[(kh, kw, ct)][:, ot * 128:(ot + 1) * 128],
                           xt_[b][ct][:, 0:256].rearrange(
                               "p (i a j b2) -> p a b2 i j",
                               i=8, a=2, j=8, b2=2)[:, kh, kw, :, :],
                           start=False, stop=False,
                           skip_group_check=True), None)
                    op("pe", lambda e, bank=bank, ot=ot, kh=kh, kw=kw,
                       ct=ct, b=b, idx=idx: nc.tensor.matmul(
                           banks[bank][:, MT:M],
                           wsr_sb[(kh, kw, ct)][:, ot * 128:(ot + 1) * 128],
                           xt_[b][ct][:, 256:1280].rearrange(
                               "p (i a j b2) -> p a b2 i j",
                               i=16, a=2, j=16, b2=2)[:, kh, kw, :, :],
                           start=False, stop=(idx == 8),
                           skip_group_check=True),
                       "pe" if idx == 8 else None)
        m = cnt["pe"]
        if ot == 1:
            pe_conv_mark[bl] = m
        wait("dve", "pe", m)
        cv = op("dve", lambda e, bank=bank, ot=ot, b=b:
                e.tensor_copy(yt_[b][ot], banks[bank][:, 0:M]), "dve")
        last_bank[bank] = ("dve", cv)
        wait("dve", "dve", cv)  # self-sync: drain yt write before read
        op("dve", lambda e, ot=ot, b=b:
           e.tensor_mul(ysq_[b][ot], yt_[b][ot], yt_[b][ot]), "dve")
        if ot == 1:
            ysq_done_mark[bl] = cnt["dve"]

    def qconv_parcels(bl):
        return ([lambda g=g: qt_parcel(bl, g) for g in range(6)]
                + [lambda ot=ot: conv_parcel(bl, ot) for ot in range(2)])

    def stats_parcel(bl):
        # LN stats + rstd, on banks 0/1 (banks 6/7 belong to attention)
        b = bl % 2
        wait("pe", "dve", ysq_done_mark[bl])
        wait_bank("pe", 0)
        wait_bank("pe", 1)
        for ot in range(2):
            op("pe", lambda e, ot=ot, b=b: nc.tensor.matmul(
                banks[0][0:1, 0:M], ones_bf[:, 0:1], yt_[b][ot],
                start=(ot == 0), stop=(ot == 1), skip_group_check=True),
               "pe" if ot == 1 else None)
        for ot in range(2):
            op("pe", lambda e, ot=ot, b=b: nc.tensor.matmul(
                banks[1][0:1, 0:M], ones_bf[:, 0:1], ysq_[b][ot],
                start=(ot == 0), stop=(ot == 1), skip_group_check=True),
               "pe" if ot == 1 else None)
        m = cnt["pe"]
        wait("dve", "pe", m)
        op("dve", lambda e, b=b: e.tensor_single_scalar(
            out=mu_[b], in_=banks[0][0:1, 0:M], scalar=1.0 / C,
            op=mybir.AluOpType.mult), "dve")
        sv = op("dve", lambda e, b=b: e.tensor_single_scalar(
            out=e2_[b], in_=banks[1][0:1, 0:M], scalar=1.0 / C,
            op=mybir.AluOpType.mult), "dve")
        last_bank[0] = ("dve", sv)
        last_bank[1] = ("dve", sv)
        wait("dve", "dve", sv)
        sv = op("dve", lambda e, b=b: e.tensor_mul(var_[b], mu_[b], mu_[b]),
                "dve")
        wait("dve", "dve", sv)
        dv_var = op("dve", lambda e, b=b:
                    e.tensor_sub(var_[b], e2_[b], var_[b]), "dve")
        wait("act", "dve", dv_var)
        av_ = op("act", lambda e, b=b: e.activation(
            lnv_[b], var_[b], LN_, bias=eps_sb[0:1, 0:1]), "act")
        wait("act", "act", av_)
        act_rstd_mark[bl] = op("act", lambda e, b=b: e.activation(
            rstd_[b], lnv_[b], EXP, scale=-0.5), "act")

    def bcast_parcel(bl):
        b = bl % 2
        wait("pe", "act", act_rstd_mark[bl])
        wait_bank("pe", 0)
        wait_bank("pe", 1)
        op("pe", lambda e, b=b: nc.tensor.matmul(
            banks[0][:, 0:M], ones_f32[0:1, :], mu_[b],
            start=True, stop=True, skip_group_check=True), "pe")
        op("pe", lambda e, b=b: nc.tensor.matmul(
            banks[1][:, 0:M], ones_f32[0:1, :], rstd_[b],
            start=True, stop=True, skip_group_check=True), "pe")
        m = cnt["pe"]
        wait("dve", "pe", m)
        for ot in range(2):
            if ot == 1:
                wait("dve", "dve", cnt["dve"])  # WAR: tmp_f reread pending
            sv = op("dve", lambda e, ot=ot, b=b:
                    e.tensor_sub(tmp_f, yt_[b][ot], banks[0][:, 0:M]), "dve")
            wait("dve", "dve", sv)
            op("dve", lambda e, ot=ot, b=b:
               e.tensor_mul(yn_[b][ot], tmp_f, banks[1][:, 0:M]), "dve")
        dvn = cnt["dve"]
        dve_norm_mark[bl] = dvn
        last_bank[0] = ("dve", dvn)
        last_bank[1] = ("dve", dvn)

    def kt_parcel(bl, dt):
        b = bl % 2
        wait("pe", "dve", dve_norm_mark[bl])
        bank = dt % 2
        wait_bank("pe", bank)
        op("pe", lambda e, bank=bank, dt=dt: nc.tensor.matmul(
            banks[bank][:, 0:M],
            uk_sb[0:1, dt * 128:(dt + 1) * 128], ones_bf[0:1, 0:M],
            start=True, stop=False, skip_group_check=True), None)
        for ct in range(2):
            op("pe", lambda e, bank=bank, dt=dt, ct=ct, b=b:
               nc.tensor.matmul(
                   banks[bank][:, 0:M],
                   wk_sb[ct][:, dt * 128:(dt + 1) * 128], yn_[b][ct],
                   start=False, stop=(ct == 1), skip_group_check=True),
               "pe" if ct == 1 else None)
        m = cnt["pe"]
        wait("dve", "pe", m)
        cv = op("dve", lambda e, bank=bank, dt=dt, b=b:
                e.tensor_copy(kt_[b][dt], banks[bank][:, 0:M]), "dve")
        last_bank[bank] = ("dve", cv)
        if dt == 1:
            kt_done_mark[bl] = cnt["dve"]

    def v_parcel(bl, mi):
        b = bl % 2
        m0, mr = MTILES[mi]
        bank = mi % 2
        wait_bank("pe", bank)
        op("pe", lambda e, bank=bank, mr=mr: nc.tensor.matmul(
            banks[bank][:mr, 0:C], ones_bf[0:1, 0:mr], uv_sb[0:1, :],
            start=True, stop=False, skip_group_check=True), None)
        for ct in range(2):
            op("pe", lambda e, bank=bank, m0=m0, mr=mr, ct=ct, b=b:
               nc.tensor.matmul(
                   banks[bank][:mr, 0:C],
                   yn_[b][ct][:, m0:m0 + mr], wv_sb[ct],
                   start=False, stop=(ct == 1), skip_group_check=True),
               "pe" if ct == 1 else None)
        m = cnt["pe"]
        wait("dve", "pe", m)
        cv = op("dve", lambda e, bank=bank, mi=mi, mr=mr, b=b:
                e.tensor_copy(v_[b][mi][:mr, :], banks[bank][:mr, 0:C]),
                "dve")
        if mi == 0:
            cv = op("dve", lambda e, bank=bank, b=b:
                    e.tensor_copy(v_[b][0][64:128, :],
                                  banks[bank][0:64, 0:C]), "dve")
        last_bank[bank] = ("dve", cv)
        if mi == 2:
            kv_done_mark[bl] = cnt["dve"]

    def kv_parcels(bl):
        return ([lambda: stats_parcel(bl), lambda: bcast_parcel(bl)]
                + [lambda dt=dt: kt_parcel(bl, dt) for dt in range(2)]
                + [lambda mi=mi: v_parcel(bl, mi) for mi in range(3)])

    emit_xt_dma(0)
    wait("pe", "dma_c", CONST_DMA)
    wait("pe", "dve", MEMSET_DONE)
    for p in qconv_parcels(0):
        p()
    parcel_q = []
    for bl in range(BL):
        b = bl % 2
        if bl == 0:
            emit_xt_dma(1)
            parcel_q.extend(qconv_parcels(1))
            for p in kv_parcels(0):
                p()
                if parcel_q:
                    parcel_q.pop(0)()
            parcel_q.extend(kv_parcels(1))
        elif bl + 1 < BL:
            emit_xt_dma(bl + 1)
            parcel_q.extend(qconv_parcels(bl + 1))
            parcel_q.extend(kv_parcels(bl + 1))
        if bl > 0:
            parcel_q.extend(proj_parcels(bl - 1))
        final_proj = proj_parcels(bl) if bl == BL - 1 else []

        # ---- attention ----
        for ci, (n0, w, is_t) in enumerate(ACH):
            for dt in range(2):
                tiles = [("T", 0), ("T", 1)]
                if not is_t:
                    tiles += [(mi, h4) for mi in (1, 2) for h4 in range(4)]
                exp_marks = []
                for ti, (kind, sub) in enumerate(tiles):
                    pi = ti // 2
                    pool = pi % 2          # 0 -> banks 2,3  1 -> banks 6,7
                    bank = (2 if pool == 0 else 6) + (ti % 2)
                    wait_bank("pe", bank)
                    if ti == 0:
                        wait("pe", "dve", kv_done_mark[bl])
                    if kind == "T":
                        pair = sub
                        for z, h4 in enumerate((2 * pair, 2 * pair + 1)):
                            op("pe", lambda e, bank=bank, h4=h4, z=z, n0=n0,
                               w=w, dt=dt, b=b: nc.tensor.matmul(
                                   banks[bank][64 * z:64 * z + 64, :w],
                                   kt_[b][dt][32 * h4:32 * h4 + 32, 0:64],
                                   qt_[b][dt][32 * h4:32 * h4 + 32, n0:n0 + w],
                                   start=True, stop=True,
                                   skip_group_check=True,
                                   tile_position=(32 * h4, 64 * z)),
                               "pe" if z == 1 else None)
                    else:
                        mi, h4 = kind, sub
                        m0m, mrm = MTILES[mi]
                        op("pe", lambda e, bank=bank, h4=h4, m0m=m0m, mrm=mrm,
                           n0=n0, w=w, dt=dt, b=b: nc.tensor.matmul(
                               banks[bank][:mrm, :w],
                               kt_[b][dt][32 * h4:32 * h4 + 32, m0m:m0m + mrm],
                               qt_[b][dt][32 * h4:32 * h4 + 32, n0:n0 + w],
                               start=True, stop=True, skip_group_check=True,
                               tile_position=(32 * h4, 0)), "pe")
                    if ti % 2 == 1:
                        # exp both tiles of the pair in one ACT instruction
                        m = cnt["pe"]
                        wait("act", "pe", m)
                        ev = op("act", lambda e, pool=pool, pi=pi, w=w, b=b:
                                e.activation(
                                    etp_[b][pi].rearrange(
                                        "p (t q) -> p t q", t=2)[:, :, 0:w],
                                    pair_of[pool].rearrange(
                                        "p (t q) -> p t q", t=2)[:, :, 0:w],
                                    EXP, scale=SCALE), "act")
                        exp_marks.append(ev)
                        lo = 2 if pool == 0 else 6
                        last_bank[lo] = ("act", ev)
                        last_bank[lo + 1] = ("act", ev)

                # AV + sums
                wait("pe", "act", exp_marks[-1])
                wait_bank("pe", 4)
                wait_bank("pe", 5)
                for h4 in range(4):
                    h = 4 * dt + h4
                    pair, lohi = divmod(h4, 2)
                    pb_ = 64 * lohi
                    ti_t = pair
                    op("pe", lambda e, h4=h4, h=h, pb_=pb_, ti_t=ti_t, w=w,
                       is_t=is_t, b=b: nc.tensor.matmul(
                           banks[4][32 * h4:32 * h4 + 32, :w],
                           v_[b][0][pb_:pb_ + 64, h * 32:h * 32 + 32],
                           et_[b][ti_t][pb_:pb_ + 64, :w],
                           start=True, stop=is_t, skip_group_check=True,
                           tile_position=(pb_, 32 * h4)), None)
                    op("pe", lambda e, h4=h4, pb_=pb_, ti_t=ti_t, w=w,
                       is_t=is_t, b=b: nc.tensor.matmul(
                           banks[5][32 * h4:32 * h4 + 32, :w],
                           ones_bf[pb_:pb_ + 64, 0:32],
                           et_[b][ti_t][pb_:pb_ + 64, :w],
                           start=True, stop=is_t, skip_group_check=True,
                           tile_position=(pb_, 32 * h4)),
                       "pe" if (is_t and h4 == 3) else None)
                    if not is_t:
                        for mi in (1, 2):
                            m0m, mrm = MTILES[mi]
                            ti_s = 2 + (mi - 1) * 4 + h4
                            op("pe", lambda e, h4=h4, h=h, mi=mi, mrm=mrm,
                               ti_s=ti_s, w=w, b=b: nc.tensor.matmul(
                                   banks[4][32 * h4:32 * h4 + 32, :w],
                                   v_[b][mi][:mrm, h * 32:h * 32 + 32],
                                   et_[b][ti_s][:mrm, :w],
                                   start=False, stop=(mi == 2),
                                   skip_group_check=True,
                                   tile_position=(0, 32 * h4)), None)
                            op("pe", lambda e, h4=h4, mi=mi, mrm=mrm,
                               ti_s=ti_s, w=w, b=b: nc.tensor.matmul(
                                   banks[5][32 * h4:32 * h4 + 32, :w],
                                   ones_bf[:mrm, 0:32],
                                   et_[b][ti_s][:mrm, :w],
                                   start=False, stop=(mi == 2),
                                   skip_group_check=True,
                                   tile_position=(0, 32 * h4)),
                               "pe" if (h4 == 3 and mi == 2) else None)
                PE_AVS = cnt["pe"]
                # Stage AV+sums out of PSUM with one wide ACT copy so banks
                # 4/5 free early; recip+mul then run off the critical path.
                u = unit_ctr[0] % 2
                unit_ctr[0] += 1
                wait("act", "pe", PE_AVS)
                if av_last[u] is not None:
                    wait("act", "dve", av_last[u])
                acv = op("act", lambda e, u=u, w=w: e.activation(
                    av_stage[u].rearrange("p (t q) -> p t q", t=2)[:, :, 0:w],
                    pairC.rearrange("p (t q) -> p t q", t=2)[:, :, 0:w],
                    mybir.ActivationFunctionType.Copy), "act")
                last_bank[4] = ("act", acv)
                last_bank[5] = ("act", acv)
                wait("dve", "act", acv)
                if rinv_last[0] is not None:
                    wait("dve", "dve", rinv_last[0])
                rv = op("dve", lambda e, u=u, w=w: e.reciprocal(
                    rinv_sb[:, :w], av_stage[u][:, 512:512 + w]), "dve")
                wait("dve", "dve", rv)
                mvv = op("dve", lambda e, u=u, dt=dt, n0=n0, w=w, b=b:
                         e.tensor_mul(ot_[b][dt][:, n0:n0 + w],
                                      av_stage[u][:, :w], rinv_sb[:, :w]),
                         "dve")
                av_last[u] = mvv
                rinv_last[0] = mvv
                if dt == 1:
                    att_chunk_mark[(bl, ci)] = cnt["dve"]
                    if bl == BL - 1 and ci == 1:
                        parcel_q.extend(final_proj[:6])

        att_done_mark[bl] = cnt["dve"]
        if bl == BL - 1:
            for p in final_proj[6:]:
                p()
        while parcel_q:
            parcel_q.pop(0)()

    # ---- emit ----
    with ExitStack() as stack:
        block = stack.enter_context(nc.Block())
        sems = {}
        for name in ["pe", "dve", "act"] + DMA_SEMS:
            sems[name] = stack.enter_context(nc.semaphore(f"{name}_sem"))

        def emit(eng_obj, items):
            for kind, a, b2 in items:
                if kind == "wait":
                    eng_obj.wait_ge(sems[a], b2)
                else:
                    ins = a(eng_obj)
                    if b2 is not None:
                        ins.then_inc(sems[b2], 16 if b2 in DMA_SEMS else 1)

        @block.sync
        def _(sync):
            emit(sync, plan["sync"])

        @block.tensor
        def _(tensor):
            emit(tensor, plan["pe"])

        @block.vector
        def _(vector):
            emit(vector, plan["dve"])

        @block.scalar
        def _(scalar):
            emit(scalar, plan["act"])

    return nc


def _get_nc():
    if "nc" not in _CACHED:
        _CACHED["nc"] = _build_nc()
    return _CACHED["nc"]


def _bf16(a):
    return np.ascontiguousarray(a.astype(ml_dtypes.bfloat16))


def kernel(x, Wq, Wkv, sr_w, sr_b, ln_g, ln_b, proj_w, proj_b,
           t_h=16, t_w=16, s_h=32, s_w=32, **_ignored):
    x = np.asarray(x, np.float32)
    Wq = np.asarray(Wq, np.float32)
    Wkv = np.asarray(Wkv, np.float32)
    sr_w = np.asarray(sr_w, np.float32)
    sr_b = np.asarray(sr_b, np.float32)
    ln_g = np.asarray(ln_g, np.float32)
    ln_b = np.asarray(ln_b, np.float32)
    proj_w = np.asarray(proj_w, np.float32)
    proj_b = np.asarray(proj_b, np.float32)

    # host-side prep: transposes + LN-gamma/beta folding (cheap, numpy)
    xT = _bf16(x.transpose(0, 2, 1))                      # [B, C, N]
    wqT = Wq.T                                            # [c, d]
    wsr = sr_w.transpose(2, 3, 1, 0)                      # [kh, kw, c, o]
    wkT = (Wkv[:C] * ln_g[None, :]).T
    wvT = (Wkv[C:] * ln_g[None, :]).T
    uk = (Wkv[:C] @ ln_b)[None, :]
    uv = (Wkv[C:] @ ln_b)[None, :]
    srb = sr_b[None, :]
    wpT = proj_w.T
    pb = proj_b[None, :]
    blocks = [wqT[:128], wqT[128:], wkT[:128], wkT[128:],
              wvT[:128], wvT[128:], wpT[:128], wpT[128:]]
    for kh in range(2):
        for kw in range(2):
            for ct in range(2):
                blocks.append(wsr[kh, kw, ct * 128:(ct + 1) * 128, :])
    blocks.append(np.tile(pb, (128, 1)))                  # proj bias tile
    wpack = _bf16(np.concatenate(blocks, axis=1))         # [128, 17*C]
    bpack = _bf16(np.concatenate([uk, uv, srb, pb], axis=1))  # [1, 4*C]

    nc = _get_nc()
    in_maps = []
    for c in range(NCORES):
        in_maps.append({
            "xT": xT[c * BL:(c + 1) * BL],
            "wpack": wpack, "bpack": bpack,
        })
    res = run_bass_kernel_spmd(nc, in_maps, core_ids=list(range(NCORES)))
    _CACHED["last_results"] = res
    outs = [np.asarray(r["out"]).astype(np.float32) for r in res.results]
    return np.concatenate(outs, axis=0)



# revision 40
# speedup vs baseline: 1.6371x; 1.0363x over previous
# Self-contained Trainium2 kernel for PVT-style sparse attention.
# reference: x[B=32,N=1280,C=256] -> q proj; conv-SR(2x2 s2)+LN on template(16x16)
# and search(32x32) tokens -> kv[320,C]; template->template-kv attention,
# search->all-kv attention; output proj. Batch-parallel over 8 NeuronCores.
import math
import sys
from contextlib import ExitStack

import numpy as np

sys.path.insert(0, "/opt/trn_rl_repo")

import ml_dtypes  # noqa: E402

import concourse.bass as bass  # noqa: E402
import concourse.mybir as mybir  # noqa: E402
import concourse.tile as tile  # noqa: E402
from concourse.bass_utils import run_bass_kernel_spmd  # noqa: E402

BF16 = mybir.dt.bfloat16
F32 = mybir.dt.float32

NCORES = 8
B = 32
BL = B // NCORES  # 4 batches per core
N = 1280
C = 256
H = 8
HD = 32
NT = 256   # template query tokens
MT = 64    # template kv tokens (8x8)
MS = 256   # search kv tokens (16x16)
M = MT + MS  # 320
SCALE = HD ** -0.5
LN_EPS = 1e-5

_CACHED = {}


def _build_nc():
    nc = bass.Bass()

    xT = nc.declare_dram_parameter("xT", [BL, C, N], BF16, isOutput=False)
    wpack = nc.declare_dram_parameter("wpack", [128, 17 * C], BF16, isOutput=False)
    bpack = nc.declare_dram_parameter("bpack", [1, 4 * C], BF16, isOutput=False)
    out = nc.declare_dram_parameter("out", [BL, N, C], F32, isOutput=True)

    EXP = mybir.ActivationFunctionType.Exp
    LN_ = mybir.ActivationFunctionType.Ln

    # ---- static SBUF ----
    def sb(name, shape, dt=BF16):
        return nc.alloc_sbuf_tensor(name, shape, dt).ap()

    wmega = sb("wmega", [128, 17 * C])
    pb_tile = wmega[:, 16 * C:17 * C]
    bmega = sb("bmega", [1, 4 * C])
    ones_bf = sb("ones_bf", [128, 512])
    ones_f32 = sb("ones_f32", [1, 128], F32)
    eps_sb = sb("eps_sb", [1, 1], F32)

    def _blk(i):
        return wmega[:, i * C:(i + 1) * C]
    wq_sb = [_blk(0), _blk(1)]
    wk_sb = [_blk(2), _blk(3)]
    wv_sb = [_blk(4), _blk(5)]
    wp_sb = [_blk(6), _blk(7)]
    wsr_sb = {(kh, kw, ct): _blk(8 + kh * 4 + kw * 2 + ct)
              for kh in range(2) for kw in range(2) for ct in range(2)}
    uk_sb = bmega[:, 0:C]
    uv_sb = bmega[:, C:2 * C]
    srb_sb = bmega[:, 2 * C:3 * C]
    pb_sb = bmega[:, 3 * C:4 * C]

    xt_ = [[sb(f"xt{b}_{ct}", [128, N]) for ct in range(2)] for b in range(2)]
    qt_ = [[sb(f"qt{b}_{dt}", [128, N]) for dt in range(2)] for b in range(2)]
    yt_ = [[sb(f"yt{b}_{o}", [128, M]) for o in range(2)] for b in range(2)]
    ysq_ = [[sb(f"ysq{b}_{o}", [128, M]) for o in range(2)] for b in range(2)]
    yn_ = [[sb(f"yn{b}_{o}", [128, M]) for o in range(2)] for b in range(2)]
    kt_ = [[sb(f"kt{b}_{d}", [128, M]) for d in range(2)] for b in range(2)]
    v_ = [[sb(f"v{b}_{m}", [128, C]) for m in range(3)] for b in range(2)]
    etp_ = [[sb(f"etp{b}_{p}", [128, 1024]) for p in range(5)]
            for b in range(2)]
    et_ = [[etp_[b][i // 2][:, (i % 2) * 512:(i % 2) * 512 + 512]
            for i in range(10)] for b in range(2)]
    ot_ = [[sb(f"ot{b}_{d}", [128, N]) for d in range(2)] for b in range(2)]
    osb_ = [sb(f"osb{j}", [128, C], F32) for j in range(4)]
    mu_ = [sb(f"mu{b}", [1, M], F32) for b in range(2)]
    e2_ = [sb(f"e2{b}", [1, M], F32) for b in range(2)]
    var_ = [sb(f"var{b}", [1, M], F32) for b in range(2)]
    lnv_ = [sb(f"lnv{b}", [1, M], F32) for b in range(2)]
    rstd_ = [sb(f"rstd{b}", [1, M], F32) for b in range(2)]
    tmp_f = sb("tmp_f", [128, M], F32)
    av_stage = [sb(f"avst{u}", [128, 1024], F32) for u in range(2)]
    rinv_sb = sb("rinv_sb", [128, 512], F32)
    rinv_tmp = sb("rinv_tmp", [128, 512], F32)

    # banks 2,3 and 6,7 are views of two-bank pair tensors so one ACT
    # instruction can exp two S^T tiles at once ([128, 2, w] AP).
    bank0 = nc.alloc_psum_tensor("bank0", [128, 512], F32).ap()
    bank1 = nc.alloc_psum_tensor("bank1", [128, 512], F32).ap()
    pairA = nc.alloc_psum_tensor("pairA", [128, 1024], F32).ap()
    pairC = nc.alloc_psum_tensor("pairC", [128, 1024], F32).ap()
    pairB = nc.alloc_psum_tensor("pairB", [128, 1024], F32).ap()
    banks = [bank0, bank1, pairA[:, 0:512], pairA[:, 512:1024],
             pairC[:, 0:512], pairC[:, 512:1024],
             pairB[:, 0:512], pairB[:, 512:1024]]
    pair_of = {0: pairA, 1: pairB}

    QCH = [(0, 512), (512, 512), (1024, 256)]
    ACH = [(0, 256, True), (256, 512, False), (768, 512, False)]
    MTILES = [(0, 64), (64, 128), (192, 128)]

    # ---- plan: four engine streams, monotonic counters ----
    # DMA completions are NOT ordered across dma_starts (multi-queue), so a
    # single shared dma counter is racy: a later DMA's +16 can satisfy a
    # wait meant for an earlier one. Each independently-consumed resource
    # gets its own semaphore; with at most K uses issued when a consumer
    # waits >= 16*K, any increments summing to the threshold imply the
    # needed DMA completed.
    DMA_SEMS = ["dma_c", "dma_x0", "dma_x1", "dma_o0", "dma_o1", "dma_o2",
                "dma_o3"]
    plan = {"sync": [], "pe": [], "dve": [], "act": []}
    cnt = {"pe": 0, "dve": 0, "act": 0}
    cnt.update({s: 0 for s in DMA_SEMS})
    last_bank = {i: None for i in range(8)}  # bank -> (sem, val) last consumer
    osb_last = {j: None for j in range(4)}

    def op(engine, fn, inc=None):
        # fn(eng) -> instruction (or None); inc sem applied via then_inc
        plan[engine].append(("op", fn, inc))
        if inc is not None:
            cnt[inc] += 16 if inc in DMA_SEMS else 1
        return cnt[inc] if inc else None

    def wait(engine, sem_name, val):
        if val is None or val <= 0:
            return
        plan[engine].append(("wait", sem_name, val))

    def wait_bank(engine, i):
        if last_bank[i] is not None:
            s, v = last_bank[i]
            if s != engine:
                wait(engine, s, v)

    # const DMAs + memsets
    op("sync", lambda e: e.dma_start(out=wmega, in_=wpack[:, :]), "dma_c")
    CONST_DMA = op("sync", lambda e: e.dma_start(out=bmega, in_=bpack[:, :]),
                   "dma_c")
    op("dve", lambda e: e.memset(ones_bf, 1.0), "dve")
    op("dve", lambda e: e.memset(ones_f32, 1.0), "dve")
    MEMSET_DONE = op("dve", lambda e: e.memset(eps_sb, LN_EPS), "dve")

    pe_conv_mark = {}
    unit_ctr = [0]
    av_last = {0: None, 1: None}
    rinv_last = [None]
    dma_xt_mark = {}
    ysq_done_mark = {}
    kv_done_mark = {}
    kt_done_mark = {}
    act_rstd_mark = {}
    dve_norm_mark = {}
    att_chunk_mark = {}
    att_done_mark = {}

    def proj_parcels(bl):
        b = bl % 2
        parcels = []

        def tile(nt, first):
            bank = nt % 2
            ci = 0 if nt < 2 else (1 if nt < 6 else 2)
            wait("pe", "dve", att_chunk_mark[(bl, ci)])
            wait_bank("pe", bank)
            for dt in range(2):
                op("pe", lambda e, bank=bank, nt=nt, dt=dt, b=b:
                   nc.tensor.matmul(
                       banks[bank][:, 0:C],
                       ot_[b][dt][:, nt * 128:(nt + 1) * 128], wp_sb[dt],
                       start=(dt == 0), stop=(dt == 1), skip_group_check=True),
                   "pe" if dt == 1 else None)
            m = cnt["pe"]
            j = nt % 4
            wait("dve", "pe", m)
            if osb_last[j] is not None:
                wait("dve", f"dma_o{j}", osb_last[j])
            # bias add fused into the PSUM->SBUF copy
            cv = op("dve", lambda e, bank=bank, j=j: e.tensor_tensor(
                out=osb_[j], in0=banks[bank][:, 0:C], in1=pb_tile,
                op=mybir.AluOpType.add), "dve")
            last_bank[bank] = ("dve", cv)
            wait("sync", "dve", cv)
            dv = op("sync", lambda e, j=j, nt=nt, bl=bl: e.dma_start(
                out=out[bl, nt * 128:(nt + 1) * 128, :], in_=osb_[j]),
                f"dma_o{j}")
            osb_last[j] = dv

        for nt in range(10):
            parcels.append(lambda nt=nt: tile(nt, nt == 0))
        return parcels

    def emit_xt_dma(bl):
        b = bl % 2
        xsem = f"dma_x{b}"
        if bl >= 2:
            wait("sync", "pe", pe_conv_mark[bl - 2])
        for ct in range(2):
            op("sync", lambda e, ct=ct, b=b, bl=bl: e.dma_start(
                out=xt_[b][ct], in_=xT[bl, ct * 128:(ct + 1) * 128, :]), xsem)
        dma_xt_mark[bl] = cnt[xsem]

    def qt_parcel(bl, g):
        b = bl % 2
        dt, (n0, w) = [(dt, ch) for dt in range(2) for ch in QCH][g]
        wait("pe", f"dma_x{b}", dma_xt_mark[bl])
        bank = g % 2
        wait_bank("pe", bank)
        for ct in range(2):
            op("pe", lambda e, bank=bank, ct=ct, dt=dt, n0=n0, w=w, b=b:
               nc.tensor.matmul(
                   banks[bank][:, :w],
                   wq_sb[ct][:, dt * 128:(dt + 1) * 128],
                   xt_[b][ct][:, n0:n0 + w],
                   start=(ct == 0), stop=(ct == 1),
                   skip_group_check=True),
               "pe" if ct == 1 else None)
        m = cnt["pe"]
        wait("dve", "pe", m)
        cv = op("dve", lambda e, bank=bank, dt=dt, n0=n0, w=w, b=b:
                e.tensor_copy(qt_[b][dt][:, n0:n0 + w], banks[bank][:, :w]),
                "dve")
        last_bank[bank] = ("dve", cv)

    def conv_parcel(bl, ot):
        b = bl % 2
        wait("pe", f"dma_x{b}", dma_xt_mark[bl])
        bank = ot % 2
        wait_bank("pe", bank)
        op("pe", lambda e, bank=bank, ot=ot: nc.tensor.matmul(
            banks[bank][:, 0:M],
            srb_sb[0:1, ot * 128:(ot + 1) * 128],
            ones_bf[0:1, 0:M],
            start=True, stop=False, skip_group_check=True), None)
        idx = 0
        for kh in range(2):
            for kw in range(2):
                for ct in range(2):
                    idx += 1
                    op("pe", lambda e, bank=bank, ot=ot, kh=kh, kw=kw,
                       ct=ct, b=b: nc.tensor.matmul(
                           banks[bank][:, 0:MT],
                           wsr_sb[(kh, kw, ct)][:, ot * 128:(ot + 1) * 128],
                           xt_[b][ct][:, 0:256].rearrange(
                               "p (i a j b2) -> p a b2 i j",
                               i=8, a=2, j=8, b2=2)[:, kh, kw, :, :],
                           start=False, stop=False,
                           skip_group_check=True), None)
                    op("pe", lambda e, bank=bank, ot=ot, kh=kh, kw=kw,
                       ct=ct, b=b, idx=idx: nc.tensor.matmul(
                           banks[bank][:, MT:M],
                           wsr_sb[(kh, kw, ct)][:, ot * 128:(ot + 1) * 128],
                           xt_[b][ct][:, 256:1280].rearrange(
                               "p (i a j b2) -> p a b2 i j",
                               i=16, a=2, j=16, b2=2)[:, kh, kw, :, :],
                           start=False, stop=(idx == 8),
                           skip_group_check=True),
                       "pe" if idx == 8 else None)
        m = cnt["pe"]
        if ot == 1:
            pe_conv_mark[bl] = m
        wait("dve", "pe", m)
        cv = op("dve", lambda e, bank=bank, ot=ot, b=b:
                e.tensor_copy(yt_[b][ot], banks[bank][:, 0:M]), "dve")
        last_bank[bank] = ("dve", cv)
        wait("dve", "dve", cv)  # self-sync: drain yt write before read
        op("dve", lambda e, ot=ot, b=b:
           e.tensor_mul(ysq_[b][ot], yt_[b][ot], yt_[b][ot]), "dve")
        if ot == 1:
            ysq_done_mark[bl] = cnt["dve"]

    def qconv_parcels(bl):
        return ([lambda g=g: qt_parcel(bl, g) for g in range(6)]
                + [lambda ot=ot: conv_parcel(bl, ot) for ot in range(2)])

    def stats_parcel(bl):
        # LN stats + rstd, on banks 0/1 (banks 6/7 belong to attention)
        b = bl % 2
        wait("pe", "dve", ysq_done_mark[bl])
        wait_bank("pe", 0)
        wait_bank("pe", 1)
        for ot in range(2):
            op("pe", lambda e, ot=ot, b=b: nc.tensor.matmul(
                banks[0][0:1, 0:M], ones_bf[:, 0:1], yt_[b][ot],
                start=(ot == 0), stop=(ot == 1), skip_group_check=True),
               "pe" if ot == 1 else None)
        for ot in range(2):
            op("pe", lambda e, ot=ot, b=b: nc.tensor.matmul(
                banks[1][0:1, 0:M], ones_bf[:, 0:1], ysq_[b][ot],
                start=(ot == 0), stop=(ot == 1), skip_group_check=True),
               "pe" if ot == 1 else None)
        m = cnt["pe"]
        wait("dve", "pe", m)
        op("dve", lambda e, b=b: e.tensor_single_scalar(
            out=mu_[b], in_=banks[0][0:1, 0:M], scalar=1.0 / C,
            op=mybir.AluOpType.mult), "dve")
        sv = op("dve", lambda e, b=b: e.tensor_single_scalar(
            out=e2_[b], in_=banks[1][0:1, 0:M], scalar=1.0 / C,
            op=mybir.AluOpType.mult), "dve")
        last_bank[0] = ("dve", sv)
        last_bank[1] = ("dve", sv)
        wait("dve", "dve", sv)
        sv = op("dve", lambda e, b=b: e.tensor_mul(var_[b], mu_[b], mu_[b]),
                "dve")
        wait("dve", "dve", sv)
        dv_var = op("dve", lambda e, b=b:
                    e.tensor_sub(var_[b], e2_[b], var_[b]), "dve")
        wait("act", "dve", dv_var)
        av_ = op("act", lambda e, b=b: e.activation(
            lnv_[b], var_[b], LN_, bias=eps_sb[0:1, 0:1]), "act")
        wait("act", "act", av_)
        act_rstd_mark[bl] = op("act", lambda e, b=b: e.activation(
            rstd_[b], lnv_[b], EXP, scale=-0.5), "act")

    def bcast_parcel(bl):
        b = bl % 2
        wait("pe", "act", act_rstd_mark[bl])
        wait_bank("pe", 0)
        wait_bank("pe", 1)
        op("pe", lambda e, b=b: nc.tensor.matmul(
            banks[0][:, 0:M], ones_f32[0:1, :], mu_[b],
            start=True, stop=True, skip_group_check=True), "pe")
        op("pe", lambda e, b=b: nc.tensor.matmul(
            banks[1][:, 0:M], ones_f32[0:1, :], rstd_[b],
            start=True, stop=True, skip_group_check=True), "pe")
        m = cnt["pe"]
        wait("dve", "pe", m)
        for ot in range(2):
            if ot == 1:
                wait("dve", "dve", cnt["dve"])  # WAR: tmp_f reread pending
            sv = op("dve", lambda e, ot=ot, b=b:
                    e.tensor_sub(tmp_f, yt_[b][ot], banks[0][:, 0:M]), "dve")
            wait("dve", "dve", sv)
            op("dve", lambda e, ot=ot, b=b:
               e.tensor_mul(yn_[b][ot], tmp_f, banks[1][:, 0:M]), "dve")
        dvn = cnt["dve"]
        dve_norm_mark[bl] = dvn
        last_bank[0] = ("dve", dvn)
        last_bank[1] = ("dve", dvn)

    def kt_parcel(bl, dt):
        b = bl % 2
        wait("pe", "dve", dve_norm_mark[bl])
        bank = dt % 2
        wait_bank("pe", bank)
        op("pe", lambda e, bank=bank, dt=dt: nc.tensor.matmul(
            banks[bank][:, 0:M],
            uk_sb[0:1, dt * 128:(dt + 1) * 128], ones_bf[0:1, 0:M],
            start=True, stop=False, skip_group_check=True), None)
        for ct in range(2):
            op("pe", lambda e, bank=bank, dt=dt, ct=ct, b=b:
               nc.tensor.matmul(
                   banks[bank][:, 0:M],
                   wk_sb[ct][:, dt * 128:(dt + 1) * 128], yn_[b][ct],
                   start=False, stop=(ct == 1), skip_group_check=True),
               "pe" if ct == 1 else None)
        m = cnt["pe"]
        wait("dve", "pe", m)
        cv = op("dve", lambda e, bank=bank, dt=dt, b=b:
                e.tensor_copy(kt_[b][dt], banks[bank][:, 0:M]), "dve")
        last_bank[bank] = ("dve", cv)
        if dt == 1:
            kt_done_mark[bl] = cnt["dve"]

    def v_parcel(bl, mi):
        b = bl % 2
        m0, mr = MTILES[mi]
        bank = mi % 2
        wait_bank("pe", bank)
        op("pe", lambda e, bank=bank, mr=mr: nc.tensor.matmul(
            banks[bank][:mr, 0:C], ones_bf[0:1, 0:mr], uv_sb[0:1, :],
            start=True, stop=False, skip_group_check=True), None)
        for ct in range(2):
            op("pe", lambda e, bank=bank, m0=m0, mr=mr, ct=ct, b=b:
               nc.tensor.matmul(
                   banks[bank][:mr, 0:C],
                   yn_[b][ct][:, m0:m0 + mr], wv_sb[ct],
                   start=False, stop=(ct == 1), skip_group_check=True),
               "pe" if ct == 1 else None)
        m = cnt["pe"]
        wait("dve", "pe", m)
        cv = op("dve", lambda e, bank=bank, mi=mi, mr=mr, b=b:
                e.tensor_copy(v_[b][mi][:mr, :], banks[bank][:mr, 0:C]),
                "dve")
        if mi == 0:
            cv = op("dve", lambda e, bank=bank, b=b:
                    e.tensor_copy(v_[b][0][64:128, :],
                                  banks[bank][0:64, 0:C]), "dve")
        last_bank[bank] = ("dve", cv)
        if mi == 2:
            kv_done_mark[bl] = cnt["dve"]

    def kv_parcels(bl):
        return ([lambda: stats_parcel(bl), lambda: bcast_parcel(bl)]
                + [lambda dt=dt: kt_parcel(bl, dt) for dt in range(2)]
                + [lambda mi=mi: v_parcel(bl, mi) for mi in range(3)])

    emit_xt_dma(0)
    wait("pe", "dma_c", CONST_DMA)
    wait("pe", "dve", MEMSET_DONE)
    for p in qconv_parcels(0):
        p()
    parcel_q = []
    for bl in range(BL):
        b = bl % 2
        if bl == 0:
            for p in kv_parcels(0):
                p()
        if bl + 1 < BL:
            emit_xt_dma(bl + 1)
            parcel_q.extend(qconv_parcels(bl + 1))
            parcel_q.extend(kv_parcels(bl + 1))
        if bl > 0:
            parcel_q.extend(proj_parcels(bl - 1))
        final_proj = proj_parcels(bl) if bl == BL - 1 else []

        # ---- attention ----
        for ci, (n0, w, is_t) in enumerate(ACH):
            for dt in range(2):
                tiles = [("T", 0), ("T", 1)]
                if not is_t:
                    tiles += [(mi, h4) for mi in (1, 2) for h4 in range(4)]
                exp_marks = []
                for ti, (kind, sub) in enumerate(tiles):
                    pi = ti // 2
                    pool = pi % 2          # 0 -> banks 2,3  1 -> banks 6,7
                    bank = (2 if pool == 0 else 6) + (ti % 2)
                    wait_bank("pe", bank)
                    if ti == 0:
                        wait("pe", "dve", kv_done_mark[bl])
                    if kind == "T":
                        pair = sub
                        for z, h4 in enumerate((2 * pair, 2 * pair + 1)):
                            op("pe", lambda e, bank=bank, h4=h4, z=z, n0=n0,
                               w=w, dt=dt, b=b: nc.tensor.matmul(
                                   banks[bank][64 * z:64 * z + 64, :w],
                                   kt_[b][dt][32 * h4:32 * h4 + 32, 0:64],
                                   qt_[b][dt][32 * h4:32 * h4 + 32, n0:n0 + w],
                                   start=True, stop=True,
                                   skip_group_check=True,
                                   tile_position=(32 * h4, 64 * z)),
                               "pe" if z == 1 else None)
                    else:
                        mi, h4 = kind, sub
                        m0m, mrm = MTILES[mi]
                        op("pe", lambda e, bank=bank, h4=h4, m0m=m0m, mrm=mrm,
                           n0=n0, w=w, dt=dt, b=b: nc.tensor.matmul(
                               banks[bank][:mrm, :w],
                               kt_[b][dt][32 * h4:32 * h4 + 32, m0m:m0m + mrm],
                               qt_[b][dt][32 * h4:32 * h4 + 32, n0:n0 + w],
                               start=True, stop=True, skip_group_check=True,
                               tile_position=(32 * h4, 0)), "pe")
                    if ti % 2 == 1:
                        # exp both tiles of the pair in one ACT instruction
                        m = cnt["pe"]
                        wait("act", "pe", m)
                        ev = op("act", lambda e, pool=pool, pi=pi, w=w, b=b:
                                e.activation(
                                    etp_[b][pi].rearrange(
                                        "p (t q) -> p t q", t=2)[:, :, 0:w],
                                    pair_of[pool].rearrange(
                                        "p (t q) -> p t q", t=2)[:, :, 0:w],
                                    EXP, scale=SCALE), "act")
                        exp_marks.append(ev)
                        lo = 2 if pool == 0 else 6
                        last_bank[lo] = ("act", ev)
                        last_bank[lo + 1] = ("act", ev)

                # AV + sums
                wait("pe", "act", exp_marks[-1])
                wait_bank("pe", 4)
                wait_bank("pe", 5)
                for h4 in range(4):
                    h = 4 * dt + h4
                    pair, lohi = divmod(h4, 2)
                    pb_ = 64 * lohi
                    ti_t = pair
                    op("pe", lambda e, h4=h4, h=h, pb_=pb_, ti_t=ti_t, w=w,
                       is_t=is_t, b=b: nc.tensor.matmul(
                           banks[4][32 * h4:32 * h4 + 32, :w],
                           v_[b][0][pb_:pb_ + 64, h * 32:h * 32 + 32],
                           et_[b][ti_t][pb_:pb_ + 64, :w],
                           start=True, stop=is_t, skip_group_check=True,
                           tile_position=(pb_, 32 * h4)), None)
                    op("pe", lambda e, h4=h4, pb_=pb_, ti_t=ti_t, w=w,
                       is_t=is_t, b=b: nc.tensor.matmul(
                           banks[5][32 * h4:32 * h4 + 32, :w],
                           ones_bf[pb_:pb_ + 64, 0:32],
                           et_[b][ti_t][pb_:pb_ + 64, :w],
                           start=True, stop=is_t, skip_group_check=True,
                           tile_position=(pb_, 32 * h4)),
                       "pe" if (is_t and h4 == 3) else None)
                    if not is_t:
                        for mi in (1, 2):
                            m0m, mrm = MTILES[mi]
                            ti_s = 2 + (mi - 1) * 4 + h4
                            op("pe", lambda e, h4=h4, h=h, mi=mi, mrm=mrm,
                               ti_s=ti_s, w=w, b=b: nc.tensor.matmul(
                                   banks[4][32 * h4:32 * h4 + 32, :w],
                                   v_[b][mi][:mrm, h * 32:h * 32 + 32],
                                   et_[b][ti_s][:mrm, :w],
                                   start=False, stop=(mi == 2),
                                   skip_group_check=True,
                                   tile_position=(0, 32 * h4)), None)
                            op("pe", lambda e, h4=h4, mi=mi, mrm=mrm,
                               ti_s=ti_s, w=w, b=b: nc.tensor.matmul(
                                   banks[5][32 * h4:32 * h4 + 32, :w],
                                   ones_bf[:mrm, 0:32],
                                   et_[b][ti_s][:mrm, :w],
                                   start=False, stop=(mi == 2),
                                   skip_group_check=True,
                                   tile_position=(0, 32 * h4)),
                               "pe" if (h4 == 3 and mi == 2) else None)
                PE_AVS = cnt["pe"]
                # Stage AV+sums out of PSUM with one wide ACT copy so banks
                # 4/5 free early; recip+mul then run off the critical path.
                u = unit_ctr[0] % 2
                unit_ctr[0] += 1
                wait("act", "pe", PE_AVS)
                if av_last[u] is not None:
                    wait("act", "dve", av_last[u])
                acv = op("act", lambda e, u=u, w=w: e.activation(
                    av_stage[u].rearrange("p (t q) -> p t q", t=2)[:, :, 0:w],
                    pairC.rearrange("p (t q) -> p t q", t=2)[:, :, 0:w],
                    mybir.ActivationFunctionType.Copy), "act")
                last_bank[4] = ("act", acv)
                last_bank[5] = ("act", acv)
                wait("dve", "act", acv)
                if rinv_last[0] is not None:
                    wait("dve", "dve", rinv_last[0])
                rv = op("dve", lambda e, u=u, w=w: e.reciprocal(
                    rinv_sb[:, :w], av_stage[u][:, 512:512 + w]), "dve")
                wait("dve", "dve", rv)
                mvv = op("dve", lambda e, u=u, dt=dt, n0=n0, w=w, b=b:
                         e.tensor_mul(ot_[b][dt][:, n0:n0 + w],
                                      av_stage[u][:, :w], rinv_sb[:, :w]),
                         "dve")
                av_last[u] = mvv
                rinv_last[0] = mvv
                if dt == 1:
                    att_chunk_mark[(bl, ci)] = cnt["dve"]
                    if bl == BL - 1 and ci == 1:
                        parcel_q.extend(final_proj[:6])

        att_done_mark[bl] = cnt["dve"]
        if bl == BL - 1:
            for p in final_proj[6:]:
                p()
        while parcel_q:
            parcel_q.pop(0)()

    # ---- emit ----
    with ExitStack() as stack:
        block = stack.enter_context(nc.Block())
        sems = {}
        for name in ["pe", "dve", "act"] + DMA_SEMS:
            sems[name] = stack.enter_context(nc.semaphore(f"{name}_sem"))

        def emit(eng_obj, items):
            for kind, a, b2 in items:
                if kind == "wait":
                    eng_obj.wait_ge(sems[a], b2)
                else:
                    ins = a(eng_obj)
                    if b2 is not None:
                        ins.then_inc(sems[b2], 16 if b2 in DMA_SEMS else 1)

        @block.sync
        def _(sync):
            emit(sync, plan["sync"])

        @block.tensor
        def _(tensor):
            emit(tensor, plan["pe"])

        @block.vector
        def _(vector):
            emit(vector, plan["dve"])

        @block.scalar
        def _(scalar):
            emit(scalar, plan["act"])

    return nc


def _get_nc():
    if "nc" not in _CACHED:
        _CACHED["nc"] = _build_nc()
    return _CACHED["nc"]


def _bf16(a):
    return np.ascontiguousarray(a.astype(ml_dtypes.bfloat16))


def kernel(x, Wq, Wkv, sr_w, sr_b, ln_g, ln_b, proj_w, proj_b,
           t_h=16, t_w=16, s_h=32, s_w=32, **_ignored):
    x = np.asarray(x, np.float32)
    Wq = np.asarray(Wq, np.float32)
    Wkv = np.asarray(Wkv, np.float32)
    sr_w = np.asarray(sr_w, np.float32)
    sr_b = np.asarray(sr_b, np.float32)
    ln_g = np.asarray(ln_g, np.float32)
    ln_b = np.asarray(ln_b, np.float32)
    proj_w = np.asarray(proj_w, np.float32)
    proj_b = np.asarray(proj_b, np.float32)

    # host-side prep: transposes + LN-gamma/beta folding (cheap, numpy)
    xT = _bf16(x.transpose(0, 2, 1))                      # [B, C, N]
    wqT = Wq.T                                            # [c, d]
    wsr = sr_w.transpose(2, 3, 1, 0)                      # [kh, kw, c, o]
    wkT = (Wkv[:C] * ln_g[None, :]).T
    wvT = (Wkv[C:] * ln_g[None, :]).T
    uk = (Wkv[:C] @ ln_b)[None, :]
    uv = (Wkv[C:] @ ln_b)[None, :]
    srb = sr_b[None, :]
    wpT = proj_w.T
    pb = proj_b[None, :]
    blocks = [wqT[:128], wqT[128:], wkT[:128], wkT[128:],
              wvT[:128], wvT[128:], wpT[:128], wpT[128:]]
    for kh in range(2):
        for kw in range(2):
            for ct in range(2):
                blocks.append(wsr[kh, kw, ct * 128:(ct + 1) * 128, :])
    blocks.append(np.tile(pb, (128, 1)))                  # proj bias tile
    wpack = _bf16(np.concatenate(blocks, axis=1))         # [128, 17*C]
    bpack = _bf16(np.concatenate([uk, uv, srb, pb], axis=1))  # [1, 4*C]

    nc = _get_nc()
    in_maps = []
    for c in range(NCORES):
        in_maps.append({
            "xT": xT[c * BL:(c + 1) * BL],
            "wpack": wpack, "bpack": bpack,
        })
    res = run_bass_kernel_spmd(nc, in_maps, core_ids=list(range(NCORES)))
    _CACHED["last_results"] = res
    outs = [np.asarray(r["out"]).astype(np.float32) for r in res.results]
    return np.concatenate(outs, axis=0)

